# revision 51
# baseline (speedup 1.0000x reference)
"""Local causal (sliding-window) attention on 8 Trainium2 NeuronCores.

Strategy: sequence-parallel, fully transposed dataflow (features on
partitions, tokens on the free dim). Each core owns 512 consecutive query
tokens of one batch element (cores 0-3 -> batch 0, cores 4-7 -> batch 1)
plus a 128-token halo whose k/v are recomputed locally, so no inter-core
communication is needed.

All matmul inputs are bfloat16 (converted on host): halves HBM traffic vs
f32, runs the 128-wide attention matmuls at full PE rate (f32r drops to 1/4
rate below 256 moving columns), and enables fast weight load. PSUM
accumulation stays f32; softmax reciprocals are computed in f32. The output
is staged to DRAM in bf16 and widened to f32 on the host.

Attention per (supertile st of 256 queries, head h): the 256-query window
spans 3 key blocks of 128 tokens; fully-masked quadrants are never computed
(half-width matmuls). Engine split per head: ScalarE does the exp (PSUM ->
bf16 SBUF) and the denominator-row pack, GpSimd applies the 0/1 window mask
and the final normalizing multiply, DVE copies raw attention out of PSUM
and runs one batched reciprocal per supertile. Denominators ride an extra
ones-column in v through the AV matmul; a one-hot-selector K=16 matmul
broadcasts each head's reciprocal row across partitions.

Scheduling: the two supertiles' 32 heads form one continuous PE pipeline;
v-projection blocks 3,4 are pushed into supertile 0's attention window and
supertile 0's output projection is woven into supertile 1's attention as a
dependency-gated wavefront, so the PE stays dense (HAM stays un-throttled)
from first weight arrival to the tail. Input DMAs split across both HWDGE
rings (sync + scalar) with per-k first blocks for a fast ramp.
"""

import sys

sys.path.insert(0, "/opt/trn_rl_repo")
import numpy as np
import ml_dtypes

BF16 = ml_dtypes.bfloat16

B, S, D = 2, 2048, 1024
H, DH = 16, 64
WINDOW = 128
NCORES = 8
SLOC = 512            # queries per core
HALO = 128
TLOC = SLOC + HALO    # 640 local tokens (halo + queries)
NST = 2               # query supertiles of 256 per core
CPB = NCORES // B     # cores per batch element

_cached = {}


def _build():
    import concourse.bacc as bacc
    import concourse.mybir as mybir
    import concourse.tile as tile

    f32 = mybir.dt.float32
    bf16 = mybir.dt.bfloat16
    AF = mybir.ActivationFunctionType

    nc = bacc.Bacc(None)
    xT_d = nc.declare_dram_parameter("xT", [128, 8 * TLOC], bf16, isOutput=False)
    # weights pre-tiled on host so every DMA reads one contiguous DRAM block
    wqkv_d = nc.declare_dram_parameter("w_qkv", [6, 128, 8 * 512], bf16, isOutput=False)
    wout_d = nc.declare_dram_parameter("w_out", [128, 8 * 1024], bf16, isOutput=False)
    mask_d = nc.declare_dram_parameter("mask", [NST, 128, 512], bf16, isOutput=False)
    sel_d = nc.declare_dram_parameter("sel", [16, 8 * 128], bf16, isOutput=False)
    eye_d = nc.declare_dram_parameter("eye", [128, 128], bf16, isOutput=False)
    out_d = nc.declare_dram_parameter("outT", [D, SLOC], bf16, isOutput=True)

    with tile.TileContext(nc) as tc:
        with (
            tc.tile_pool(name="sb", bufs=1) as sb,
            tc.tile_pool(name="qkps", bufs=1, space="PSUM") as qkps,
            tc.tile_pool(name="scps", bufs=1, space="PSUM") as scps,
            tc.tile_pool(name="aops", bufs=1, space="PSUM") as aops,
            tc.tile_pool(name="pops", bufs=1, space="PSUM") as pops,
        ):
            # ---- resident SBUF tensors; DMA order = consumption order.
            # Ring split: bulk weights + outputs ride the scalar-issued HWDGE
            # ring, x/q-ramp/k/v weights ride the sync ring.
            wqb = [sb.tile([128, 8 * 512], bf16, tag=f"wqb{cb}", name=f"wqb{cb}")
                   for cb in range(6)]
            xta = sb.tile([128, 8 * TLOC], bf16, tag="xta", name="xta")
            xt = [xta[:, k * TLOC:(k + 1) * TLOC] for k in range(8)]

            # two HWDGE rings, each ordered by first-consumption time:
            # sync: q block 0 (halved for a faster k-loop start) then k block
            # cb2; scalar: x, v weights, mask/eye, then the attention-window
            # filler blocks and the out-proj weights
            nc.sync.dma_start(out=wqb[0][:, 0:4 * 512], in_=wqkv_d[0][:, 0:4 * 512])
            nc.sync.dma_start(out=wqb[0][:, 4 * 512:], in_=wqkv_d[0][:, 4 * 512:])
            nc.sync.dma_start(out=wqb[2][:], in_=wqkv_d[2])
            msk = [sb.tile([128, 512], bf16, tag=f"mk{i}", name=f"mk{i}") for i in range(NST)]
            sel = sb.tile([16, 8 * 128], bf16, tag="sel", name="sel")
            eye = sb.tile([128, 128], bf16, tag="eye", name="eye")
            wo = sb.tile([128, 8 * 1024], bf16, tag="wo", name="wo")
            nc.scalar.dma_start(out=xta[:], in_=xT_d[:])
            nc.scalar.dma_start(out=wqb[4][:], in_=wqkv_d[4])
            nc.scalar.dma_start(out=wqb[5][:], in_=wqkv_d[5])
            nc.scalar.dma_start(out=eye[:], in_=eye_d[:])
            for st in range(NST):
                nc.scalar.dma_start(out=msk[st][:], in_=mask_d[st])
            nc.scalar.dma_start(out=wqb[3][:], in_=wqkv_d[3])
            nc.scalar.dma_start(out=wqb[1][:], in_=wqkv_d[1])
            nc.scalar.dma_start(out=sel[:], in_=sel_d[:])
            nc.scalar.dma_start(out=wo[:], in_=wout_d[:])

            qT = [sb.tile([128, SLOC], bf16, tag=f"qT{i}", name=f"qT{i}") for i in range(8)]
            kT = [sb.tile([128, TLOC], bf16, tag=f"kT{i}", name=f"kT{i}") for i in range(8)]
            vt = [sb.tile([128, 65 * H], bf16, tag=f"v{t}", name=f"v{t}") for t in range(5)]
            att = [sb.tile([128, SLOC], bf16, tag=f"at{t}", name=f"at{t}") for t in range(8)]
            for t in range(5):
                nc.vector.memset(vt[t].rearrange("p (h c) -> p h c", c=65)[:, :, 64], 1.0)
            scat = [sb.tile([1, H * 256], f32, tag=f"scat{st}", name=f"scat{st}") for st in range(NST)]
            s16 = [sb.tile([16, 256], f32, tag=f"s16_{st}", name=f"s16_{st}") for st in range(NST)]
            r16f = [sb.tile([16, 256], f32, tag=f"r16f_{st}", name=f"r16f_{st}") for st in range(NST)]
            r16b = [sb.tile([16, 256], bf16, tag=f"r16b_{st}", name=f"r16b_{st}") for st in range(NST)]

            # ---- phase 1: qkv projection. Half the blocks (q cb1, k cb3,
            # v t3/t4) are deferred into supertile 0's attention window as
            # dense PE filler — the array stays busy enough to hold the HAM
            # clock-gate open while ScalarE/DVE pace the softmax pipeline ----
            def emit_q_group(cb, m, eng):
                ps = qkps.tile([128, 512], f32, tag="qk", bufs=2, name=f"psq{cb}_{m}")
                for k in range(8):
                    nc.tensor.matmul(
                        ps[:], wqb[cb][:, k * 512 + m * 128:k * 512 + (m + 1) * 128],
                        xt[k][:, HALO:TLOC],
                        start=(k == 0), stop=(k == 7),
                    )
                if eng == "v":
                    nc.vector.tensor_copy(qT[cb * 4 + m][:], ps[:])
                else:
                    nc.scalar.copy(qT[cb * 4 + m][:], ps[:])

            def emit_k_group(cb, m, n, eng):
                ps = qkps.tile([128, 320], f32, tag="qk", bufs=2, name=f"psk{cb}_{m}_{n}")
                for k in range(8):
                    nc.tensor.matmul(
                        ps[:], wqb[cb][:, k * 512 + m * 128:k * 512 + (m + 1) * 128],
                        xt[k][:, n * 320:(n + 1) * 320],
                        start=(k == 0), stop=(k == 7),
                    )
                if eng == "v":
                    nc.vector.tensor_copy(kT[(cb - 2) * 4 + m][:, n * 320:(n + 1) * 320], ps[:])
                else:
                    nc.scalar.copy(kT[(cb - 2) * 4 + m][:, n * 320:(n + 1) * 320], ps[:])

            for m in range(4):             # q cb0 -> qT[0..3]
                emit_q_group(0, m, "v")
            for m in range(4):             # k cb2 -> kT[0..3]
                for n in range(2):
                    emit_k_group(2, m, n, "v")

            def emit_v(t, half, eng):
                # x block stationary so tokens land on partitions
                ps = qkps.tile([128, 512], f32, tag="qk", bufs=2, name=f"psv{t}_{half}")
                for k in range(8):
                    nc.tensor.matmul(
                        ps[:], xt[k][:, t * 128:(t + 1) * 128],
                        wqb[4 + half][:, k * 512:(k + 1) * 512],
                        start=(k == 0), stop=(k == 7),
                    )
                h0 = half * 8
                dst = vt[t].rearrange("p (h c) -> p h c", c=65)[:, h0:h0 + 8, 0:64]
                src = ps[:].rearrange("p (h c) -> p h c", c=64)
                if eng == "v":
                    nc.vector.tensor_copy(dst, src)
                else:
                    nc.scalar.copy(dst, src)

            for t in range(3):
                for half in range(2):
                    emit_v(t, half, "v")

            # ---- phase 2+3: attention + interleaved output projection ----
            DEPTH = 3
            pend = {}

            def emit_qk(st, h):
                t, poff = h // 2, (h % 2) * 64
                jb, q0 = st * 2, st * 256
                sc = scps.tile([128, 512], f32, tag="sc", bufs=2, name=f"sc_{st}_{h}")
                nc.tensor.matmul(
                    sc[:], eye[:], msk[st][:],
                    start=True, stop=False, skip_group_check=True,
                )
                nc.tensor.matmul(
                    sc[:, 0:128],
                    kT[t][poff:poff + 64, jb * 128:(jb + 1) * 128],
                    qT[t][poff:poff + 64, q0:q0 + 128],
                    start=False, stop=False, skip_group_check=True,
                )
                nc.tensor.matmul(
                    sc[:, 128:256],
                    kT[t][poff:poff + 64, (jb + 2) * 128:(jb + 3) * 128],
                    qT[t][poff:poff + 64, q0 + 128:q0 + 256],
                    start=False, stop=False, skip_group_check=True,
                )
                nc.tensor.matmul(
                    sc[:, 256:512],
                    kT[t][poff:poff + 64, (jb + 1) * 128:(jb + 2) * 128],
                    qT[t][poff:poff + 64, q0:q0 + 256],
                    start=False, stop=True, skip_group_check=True,
                )
                p = sb.tile([128, 512], bf16, tag="pp", bufs=DEPTH + 3, name=f"p_{st}_{h}")
                nc.scalar.activation(p[:], sc[:], AF.Exp, scale=0.125)
                pend[(st, h)] = p

            def emit_av(st, h):
                t, poff = h // 2, (h % 2) * 64
                jb, q0 = st * 2, st * 256
                p = pend.pop((st, h))
                av = aops.tile([65, 256], f32, tag="ao", bufs=2, name=f"av{st}_{h}")
                nc.tensor.matmul(
                    av[:], vt[jb + 1][:, h * 65:h * 65 + 65], p[:, 256:512],
                    start=True, stop=False, skip_group_check=True,
                )
                nc.tensor.matmul(
                    av[:, 0:128], vt[jb][:, h * 65:h * 65 + 65], p[:, 0:128],
                    start=False, stop=False, skip_group_check=True,
                )
                nc.tensor.matmul(
                    av[:, 128:256], vt[jb + 2][:, h * 65:h * 65 + 65], p[:, 128:256],
                    start=False, stop=True, skip_group_check=True,
                )
                nc.scalar.copy(scat[st][0:1, h * 256:(h + 1) * 256], av[64:65, :])
                nc.sync.dma_start(
                    out=s16[st][h:h + 1, :], in_=scat[st][0:1, h * 256:(h + 1) * 256]
                )
                nc.vector.tensor_copy(att[t][poff:poff + 64, q0:q0 + 256], av[0:64, :])

            def emit_recip(st):
                # ~18 correct bits, 5x faster than reciprocal(); the bf16
                # cast below keeps only 8 bits anyway
                nc.vector.reciprocal_approx_fast(r16f[st][:], s16[st][:])
                nc.scalar.copy(r16b[st][:], r16f[st][:])

            def emit_norm_pair(st, t):
                # one selector matmul broadcasts head 2t's reciprocal onto
                # partitions 0-63 and head 2t+1's onto 64-127, so the whole
                # att tile normalizes in a single DVE multiply
                q0 = st * 256
                rb = qkps.tile([128, 256], f32, tag="qk", bufs=2, name=f"rb{st}_{t}")
                nc.tensor.matmul(
                    rb[:], sel[:, t * 128:(t + 1) * 128], r16b[st][:],
                    start=True, stop=True, skip_group_check=True,
                )
                asl = att[t][:, q0:q0 + 256]
                nc.vector.tensor_mul(asl, asl, rb[:])

            po_tile = {}
            ot_box = [None]

            def emit_po_unit(st, m, k):
                q0 = st * 256
                if k == 0:
                    po_tile[(st, m)] = pops.tile(
                        [128, 256], f32, tag="po", bufs=2, name=f"po{st}_{m}"
                    )
                po = po_tile[(st, m)]
                nc.tensor.matmul(
                    po[:], wo[:, k * 1024 + m * 128:k * 1024 + (m + 1) * 128],
                    att[k][:, q0:q0 + 256],
                    start=(k == 0), stop=(k == 7), skip_group_check=True,
                )
                if k == 7:
                    if m % 2 == 0:
                        ot_box[0] = sb.tile([128, 512], bf16, tag="ot", bufs=3, name=f"ot{st}_{m}")
                        nc.scalar.copy(ot_box[0][:, 0:256], po[:])
                    else:
                        ot = ot_box[0]
                        nc.scalar.copy(ot[:, 256:512], po[:])
                        eng = nc.sync if st == 0 else nc.scalar
                        eng.dma_start(
                            out=out_d.rearrange("(m p) q -> p m q", m=8)[:, m - 1:m + 1, q0:q0 + 256],
                            in_=ot.rearrange("p (m q) -> p m q", m=2),
                        )

            # unified pipeline: 32 heads; st0 norms + st0 out-proj wavefront
            # and deferred v blocks 3,4 fill supertile boundaries. st0's last
            # two out-proj columns are held back as PE filler spanning the
            # st1 reciprocal, so the PE never idles into a HAM re-throttle.
            po_queue = [(m, k) for m in range(5) for k in range(8)]
            po_ptr = 0
            norm_emitted = -1
            # deferred phase-1 fillers: kT[4+m] must land before QK(0, 8+2m)
            # reads it (step 8+2m), qT[4+m] likewise — both comfortably ahead
            fillers = {
                1: lambda: (emit_k_group(3, 0, 0, "v"), emit_k_group(3, 0, 1, "v")),
                2: lambda: (emit_k_group(3, 1, 0, "v"), emit_k_group(3, 1, 1, "v")),
                3: lambda: (emit_k_group(3, 2, 0, "v"), emit_k_group(3, 2, 1, "v")),
                4: lambda: (emit_k_group(3, 3, 0, "v"), emit_k_group(3, 3, 1, "v")),
                5: lambda: emit_q_group(1, 0, "v"),
                6: lambda: emit_q_group(1, 1, "v"),
                7: lambda: emit_q_group(1, 2, "v"),
                8: lambda: emit_q_group(1, 3, "v"),
                9: lambda: emit_v(3, 0, "v"),
                10: lambda: emit_v(3, 1, "v"),
                11: lambda: emit_v(4, 0, "v"),
                12: lambda: emit_v(4, 1, "v"),
            }
            for step in range(2 * H + DEPTH):
                if step in fillers:
                    fillers[step]()
                if step < 2 * H:
                    emit_qk(step // H, step % H)
                s = step - DEPTH
                if s >= 0:
                    emit_av(s // H, s % H)
                    if s == H - 1:
                        emit_recip(0)
                ns = step - (H + DEPTH)
                if 0 <= ns < 8:
                    emit_norm_pair(0, ns)
                    norm_emitted = ns
                if step > H + DEPTH:
                    drained = 0
                    while po_ptr < 40 and drained < 6:
                        m, k = po_queue[po_ptr]
                        if k <= norm_emitted:
                            emit_po_unit(0, m, k)
                            po_ptr += 1
                            drained += 1
                        else:
                            break
            while po_ptr < 40:
                emit_po_unit(0, *po_queue[po_ptr])
                po_ptr += 1
            emit_recip(1)
            for m in (5, 6, 7):               # PE filler while DVE runs recip(1)
                for k in range(8):
                    emit_po_unit(0, m, k)
            po_queue = [(m, k) for m in range(8) for k in range(8)]
            po_ptr = 0
            for j in range(8):
                emit_norm_pair(1, j)
                while po_ptr < 64:
                    m, k = po_queue[po_ptr]
                    if k <= j:
                        emit_po_unit(1, m, k)
                        po_ptr += 1
                    else:
                        break
            while po_ptr < 64:
                emit_po_unit(1, *po_queue[po_ptr])
                po_ptr += 1

    nc.finalize()
    return nc


def _get_nc():
    if "nc" not in _cached:
        _cached["nc"] = _build()
    return _cached["nc"]


def _core_inputs(x, w_qkv, w_out):
    # pre-tile weights so each on-device DMA is one contiguous DRAM block:
    # w_qkv -> [cb, p, k*512+c] with w[k*128+p, cb*512+c];
    # w_out -> [p, k*1024+c] with w[k*128+p, c]
    wq_b = np.ascontiguousarray(
        w_qkv.astype(BF16).reshape(8, 128, 6, 512).transpose(2, 1, 0, 3).reshape(6, 128, 8 * 512)
    )
    wo_b = np.ascontiguousarray(
        w_out.astype(BF16).reshape(8, 128, 1024).transpose(1, 0, 2).reshape(128, 8 * 1024)
    )
    # pair selector: col-block t broadcasts head 2t onto partitions 0-63 and
    # head 2t+1 onto partitions 64-127
    sel = np.zeros((16, 8 * 128), dtype=BF16)
    for t in range(8):
        sel[2 * t, t * 128:t * 128 + 64] = 1
        sel[2 * t + 1, t * 128 + 64:(t + 1) * 128] = 1
    in_maps = []
    for c in range(NCORES):
        b, qs = c // CPB, (c % CPB) * SLOC
        xs = np.zeros((TLOC, D), dtype=np.float32)
        lo = max(0, qs - HALO)
        xs[HALO - (qs - lo):] = x[b, lo:qs + SLOC]
        # multiplicative 0/1 mask applied to exp(scores) on GpSimd.
        # mask[st][:, 0:128] covers [r0 x queries 0:128], [:, 128:256] covers
        # [r2 x queries 128:256], [:, 256:512] is r1 for all 256 queries.
        i = np.arange(256)[None, None, None, :]
        j = np.arange(128)[None, None, :, None]
        st = np.arange(NST)[:, None, None, None]
        r = np.arange(3)[None, :, None, None]
        qg = qs + st * 256 + i
        kg = qs + st * 256 - HALO + r * 128 + j
        allowed = (kg <= qg) & (kg > qg - WINDOW) & (kg >= 0)
        # additive bias on raw scores (exp applies the 0.125 scale)
        m3 = np.where(allowed, 0.0, -8e30).astype(np.float32)
        mask = np.empty((NST, 128, 512), dtype=np.float32)
        mask[:, :, 0:128] = m3[:, 0, :, 0:128]
        mask[:, :, 128:256] = m3[:, 2, :, 128:256]
        mask[:, :, 256:512] = m3[:, 1]
        # x tiled like the weights: one contiguous [128, 8*640] block
        xtl = np.ascontiguousarray(
            xs.T.astype(BF16).reshape(8, 128, TLOC).transpose(1, 0, 2).reshape(128, 8 * TLOC)
        )
        in_maps.append(
            {
                "xT": xtl,
                "w_qkv": wq_b,
                "w_out": wo_b,
                "mask": mask.astype(BF16),
                "sel": sel,
                "eye": np.eye(128, dtype=BF16),
            }
        )
    return in_maps


def kernel(x, w_qkv, w_out, _trace=False, _trace_kwargs=None):
    from concourse.bass_utils import run_bass_kernel_spmd

    x = np.asarray(x, dtype=np.float32)
    w_qkv = np.asarray(w_qkv, dtype=np.float32)
    w_out = np.asarray(w_out, dtype=np.float32)
    nc = _get_nc()
    in_maps = _core_inputs(x, w_qkv, w_out)
    res = run_bass_kernel_spmd(
        nc, in_maps, list(range(NCORES)), trace=_trace, **(_trace_kwargs or {})
    )
    out = np.concatenate(
        [res.results[c]["outT"].astype(np.float32).T for c in range(NCORES)], axis=0
    ).reshape(B, S, D)
    if _trace:
        return out, res
    return out


# revision 53
# speedup vs baseline: 1.1110x; 1.1110x over previous
"""Local causal (sliding-window) attention on 8 Trainium2 NeuronCores.

Strategy: sequence-parallel, fully transposed dataflow (features on
partitions, tokens on the free dim). Each core owns 512 consecutive query
tokens of one batch element (cores 0-3 -> batch 0, cores 4-7 -> batch 1)
plus a 128-token halo whose k/v are recomputed locally, so no inter-core
communication is needed.

All matmul inputs are bfloat16 (converted on host): halves HBM traffic vs
f32, runs the 128-wide attention matmuls at full PE rate (f32r drops to 1/4
rate below 256 moving columns), and enables fast weight load. PSUM
accumulation stays f32; softmax reciprocals are computed in f32. The output
is staged to DRAM in bf16 and widened to f32 on the host.

Attention per (supertile st of 256 queries, head h): the 256-query window
spans 3 key blocks of 128 tokens; fully-masked quadrants are never computed
(half-width matmuls). Engine split per head: ScalarE does the exp (PSUM ->
bf16 SBUF) and the denominator-row pack, GpSimd applies the 0/1 window mask
and the final normalizing multiply, DVE copies raw attention out of PSUM
and runs one batched reciprocal per supertile. Denominators ride an extra
ones-column in v through the AV matmul; a one-hot-selector K=16 matmul
broadcasts each head's reciprocal row across partitions.

Scheduling: the two supertiles' 32 heads form one continuous PE pipeline;
v-projection blocks 3,4 are pushed into supertile 0's attention window and
supertile 0's output projection is woven into supertile 1's attention as a
dependency-gated wavefront, so the PE stays dense (HAM stays un-throttled)
from first weight arrival to the tail. Input DMAs split across both HWDGE
rings (sync + scalar) with per-k first blocks for a fast ramp.
"""

import sys

sys.path.insert(0, "/opt/trn_rl_repo")
import numpy as np
import ml_dtypes

BF16 = ml_dtypes.bfloat16

B, S, D = 2, 2048, 1024
H, DH = 16, 64
WINDOW = 128
NCORES = 8
SLOC = 512            # queries per core
HALO = 128
TLOC = SLOC + HALO    # 640 local tokens (halo + queries)
NST = 2               # query supertiles of 256 per core
CPB = NCORES // B     # cores per batch element

_cached = {}


def _build():
    import concourse.bacc as bacc
    import concourse.mybir as mybir
    import concourse.tile as tile

    f32 = mybir.dt.float32
    bf16 = mybir.dt.bfloat16
    AF = mybir.ActivationFunctionType

    nc = bacc.Bacc(None)
    xT_d = nc.declare_dram_parameter("xT", [D, TLOC], bf16, isOutput=False)
    # weights pre-tiled on host so every DMA reads one contiguous DRAM block
    wqkv_d = nc.declare_dram_parameter("w_qkv", [6, 128, 8 * 512], bf16, isOutput=False)
    wout_d = nc.declare_dram_parameter("w_out", [128, 8 * 1024], bf16, isOutput=False)
    mask_d = nc.declare_dram_parameter("mask", [NST, 128, 512], bf16, isOutput=False)
    sel_d = nc.declare_dram_parameter("sel", [16, 16 * 128], bf16, isOutput=False)
    eye_d = nc.declare_dram_parameter("eye", [128, 128], bf16, isOutput=False)
    out_d = nc.declare_dram_parameter("outT", [D, SLOC], bf16, isOutput=True)

    with tile.TileContext(nc) as tc:
        with (
            tc.tile_pool(name="sb", bufs=1) as sb,
            tc.tile_pool(name="qkps", bufs=1, space="PSUM") as qkps,
            tc.tile_pool(name="scps", bufs=1, space="PSUM") as scps,
            tc.tile_pool(name="aops", bufs=1, space="PSUM") as aops,
            tc.tile_pool(name="pops", bufs=1, space="PSUM") as pops,
        ):
            # ---- resident SBUF tensors; DMA order = consumption order.
            # Ring split: bulk weights + outputs ride the scalar-issued HWDGE
            # ring, x/q-ramp/k/v weights ride the sync ring.
            wqb = [sb.tile([128, 8 * 512], bf16, tag=f"wqb{cb}", name=f"wqb{cb}")
                   for cb in range(6)]
            xt = [sb.tile([128, TLOC], bf16, tag=f"xt{k}", name=f"xt{k}") for k in range(8)]

            # two HWDGE rings, each ordered by first-consumption time:
            # sync: q block 0 (halved for a faster k-loop start) then k block
            # cb2; scalar: x, v weights, mask/eye, then the attention-window
            # filler blocks and the out-proj weights
            nc.sync.dma_start(out=wqb[0][:, 0:4 * 512], in_=wqkv_d[0][:, 0:4 * 512])
            nc.sync.dma_start(out=wqb[0][:, 4 * 512:], in_=wqkv_d[0][:, 4 * 512:])
            nc.sync.dma_start(out=wqb[2][:], in_=wqkv_d[2])
            msk = [sb.tile([128, 512], bf16, tag=f"mk{i}", name=f"mk{i}") for i in range(NST)]
            sel = sb.tile([16, 16 * 128], bf16, tag="sel", name="sel")
            eye = sb.tile([128, 128], bf16, tag="eye", name="eye")
            wo = sb.tile([128, 8 * 1024], bf16, tag="wo", name="wo")
            for k in range(8):
                nc.scalar.dma_start(out=xt[k][:], in_=xT_d[k * 128:(k + 1) * 128, :])
            nc.scalar.dma_start(out=wqb[4][:], in_=wqkv_d[4])
            nc.scalar.dma_start(out=wqb[5][:], in_=wqkv_d[5])
            nc.scalar.dma_start(out=eye[:], in_=eye_d[:])
            for st in range(NST):
                nc.scalar.dma_start(out=msk[st][:], in_=mask_d[st])
            nc.scalar.dma_start(out=wqb[3][:], in_=wqkv_d[3])
            nc.scalar.dma_start(out=wqb[1][:], in_=wqkv_d[1])
            nc.scalar.dma_start(out=sel[:], in_=sel_d[:])
            nc.scalar.dma_start(out=wo[:], in_=wout_d[:])

            qT = [sb.tile([128, SLOC], bf16, tag=f"qT{i}", name=f"qT{i}") for i in range(8)]
            kT = [sb.tile([128, TLOC], bf16, tag=f"kT{i}", name=f"kT{i}") for i in range(8)]
            vt = [sb.tile([128, 65 * H], bf16, tag=f"v{t}", name=f"v{t}") for t in range(5)]
            att = [sb.tile([128, SLOC], bf16, tag=f"at{t}", name=f"at{t}") for t in range(8)]
            for t in range(5):
                nc.vector.memset(vt[t].rearrange("p (h c) -> p h c", c=65)[:, :, 64], 1.0)
            scat = [sb.tile([1, H * 256], f32, tag=f"scat{st}", name=f"scat{st}") for st in range(NST)]
            s16 = [sb.tile([16, 256], f32, tag=f"s16_{st}", name=f"s16_{st}") for st in range(NST)]
            r16f = [sb.tile([16, 256], f32, tag=f"r16f_{st}", name=f"r16f_{st}") for st in range(NST)]
            r16b = [sb.tile([16, 256], bf16, tag=f"r16b_{st}", name=f"r16b_{st}") for st in range(NST)]

            # ---- phase 1: qkv projection. Half the blocks (q cb1, k cb3,
            # v t3/t4) are deferred into supertile 0's attention window as
            # dense PE filler — the array stays busy enough to hold the HAM
            # clock-gate open while ScalarE/DVE pace the softmax pipeline ----
            def emit_q_group(cb, m, eng):
                ps = qkps.tile([128, 512], f32, tag="qk", bufs=2, name=f"psq{cb}_{m}")
                for k in range(8):
                    nc.tensor.matmul(
                        ps[:], wqb[cb][:, k * 512 + m * 128:k * 512 + (m + 1) * 128],
                        xt[k][:, HALO:TLOC],
                        start=(k == 0), stop=(k == 7),
                    )
                if eng == "v":
                    nc.vector.tensor_copy(qT[cb * 4 + m][:], ps[:])
                else:
                    nc.scalar.copy(qT[cb * 4 + m][:], ps[:])

            def emit_k_group(cb, m, n, eng):
                ps = qkps.tile([128, 320], f32, tag="qk", bufs=2, name=f"psk{cb}_{m}_{n}")
                for k in range(8):
                    nc.tensor.matmul(
                        ps[:], wqb[cb][:, k * 512 + m * 128:k * 512 + (m + 1) * 128],
                        xt[k][:, n * 320:(n + 1) * 320],
                        start=(k == 0), stop=(k == 7),
                    )
                if eng == "v":
                    nc.vector.tensor_copy(kT[(cb - 2) * 4 + m][:, n * 320:(n + 1) * 320], ps[:])
                else:
                    nc.scalar.copy(kT[(cb - 2) * 4 + m][:, n * 320:(n + 1) * 320], ps[:])

            for m in range(4):             # q cb0 -> qT[0..3]
                emit_q_group(0, m, "v")
            for m in range(4):             # k cb2 -> kT[0..3]
                for n in range(2):
                    emit_k_group(2, m, n, "v")

            def emit_v(t, half, eng):
                # x block stationary so tokens land on partitions
                ps = qkps.tile([128, 512], f32, tag="qk", bufs=2, name=f"psv{t}_{half}")
                for k in range(8):
                    nc.tensor.matmul(
                        ps[:], xt[k][:, t * 128:(t + 1) * 128],
                        wqb[4 + half][:, k * 512:(k + 1) * 512],
                        start=(k == 0), stop=(k == 7),
                    )
                h0 = half * 8
                dst = vt[t].rearrange("p (h c) -> p h c", c=65)[:, h0:h0 + 8, 0:64]
                src = ps[:].rearrange("p (h c) -> p h c", c=64)
                if eng == "v":
                    nc.vector.tensor_copy(dst, src)
                else:
                    nc.scalar.copy(dst, src)

            for t in range(3):
                for half in range(2):
                    emit_v(t, half, "v")

            # ---- phase 2+3: attention + interleaved output projection ----
            DEPTH = 3
            pend = {}

            def emit_qk(st, h):
                t, poff = h // 2, (h % 2) * 64
                jb, q0 = st * 2, st * 256
                sc = scps.tile([128, 512], f32, tag="sc", bufs=2, name=f"sc_{st}_{h}")
                nc.tensor.matmul(
                    sc[:], eye[:], msk[st][:],
                    start=True, stop=False, skip_group_check=True,
                )
                nc.tensor.matmul(
                    sc[:, 0:128],
                    kT[t][poff:poff + 64, jb * 128:(jb + 1) * 128],
                    qT[t][poff:poff + 64, q0:q0 + 128],
                    start=False, stop=False, skip_group_check=True,
                )
                nc.tensor.matmul(
                    sc[:, 128:256],
                    kT[t][poff:poff + 64, (jb + 2) * 128:(jb + 3) * 128],
                    qT[t][poff:poff + 64, q0 + 128:q0 + 256],
                    start=False, stop=False, skip_group_check=True,
                )
                nc.tensor.matmul(
                    sc[:, 256:512],
                    kT[t][poff:poff + 64, (jb + 1) * 128:(jb + 2) * 128],
                    qT[t][poff:poff + 64, q0:q0 + 256],
                    start=False, stop=True, skip_group_check=True,
                )
                p = sb.tile([128, 512], bf16, tag="pp", bufs=DEPTH + 3, name=f"p_{st}_{h}")
                nc.scalar.activation(p[:], sc[:], AF.Exp, scale=0.125)
                pend[(st, h)] = p

            def emit_av(st, h):
                t, poff = h // 2, (h % 2) * 64
                jb, q0 = st * 2, st * 256
                p = pend.pop((st, h))
                av = aops.tile([65, 256], f32, tag="ao", bufs=2, name=f"av{st}_{h}")
                nc.tensor.matmul(
                    av[:], vt[jb + 1][:, h * 65:h * 65 + 65], p[:, 256:512],
                    start=True, stop=False, skip_group_check=True,
                )
                nc.tensor.matmul(
                    av[:, 0:128], vt[jb][:, h * 65:h * 65 + 65], p[:, 0:128],
                    start=False, stop=False, skip_group_check=True,
                )
                nc.tensor.matmul(
                    av[:, 128:256], vt[jb + 2][:, h * 65:h * 65 + 65], p[:, 128:256],
                    start=False, stop=True, skip_group_check=True,
                )
                nc.scalar.copy(scat[st][0:1, h * 256:(h + 1) * 256], av[64:65, :])
                nc.sync.dma_start(
                    out=s16[st][h:h + 1, :], in_=scat[st][0:1, h * 256:(h + 1) * 256]
                )
                nc.vector.tensor_copy(att[t][poff:poff + 64, q0:q0 + 256], av[0:64, :])

            def emit_recip(st):
                # ~18 correct bits, 5x faster than reciprocal(); the bf16
                # cast below keeps only 8 bits anyway
                nc.vector.reciprocal_approx_fast(r16f[st][:], s16[st][:])
                nc.scalar.copy(r16b[st][:], r16f[st][:])

            def emit_norm(st, h):
                t, poff = h // 2, (h % 2) * 64
                q0 = st * 256
                rb = qkps.tile([128, 256], f32, tag="qk", bufs=2, name=f"rb{st}_{h}")
                nc.tensor.matmul(
                    rb[:], sel[:, h * 128:(h + 1) * 128], r16b[st][:],
                    start=True, stop=True, skip_group_check=True,
                )
                asl = att[t][poff:poff + 64, q0:q0 + 256]
                nc.vector.tensor_mul(asl, asl, rb[poff:poff + 64, :])

            po_tile = {}
            ot_box = [None]

            def emit_po_unit(st, m, k):
                q0 = st * 256
                if k == 0:
                    po_tile[(st, m)] = pops.tile(
                        [128, 256], f32, tag="po", bufs=2, name=f"po{st}_{m}"
                    )
                po = po_tile[(st, m)]
                nc.tensor.matmul(
                    po[:], wo[:, k * 1024 + m * 128:k * 1024 + (m + 1) * 128],
                    att[k][:, q0:q0 + 256],
                    start=(k == 0), stop=(k == 7), skip_group_check=True,
                )
                if k == 7:
                    if m % 2 == 0:
                        ot_box[0] = sb.tile([128, 512], bf16, tag="ot", bufs=3, name=f"ot{st}_{m}")
                        nc.scalar.copy(ot_box[0][:, 0:256], po[:])
                    else:
                        ot = ot_box[0]
                        nc.scalar.copy(ot[:, 256:512], po[:])
                        eng = nc.sync if st == 0 else nc.scalar
                        eng.dma_start(
                            out=out_d.rearrange("(m p) q -> p m q", m=8)[:, m - 1:m + 1, q0:q0 + 256],
                            in_=ot.rearrange("p (m q) -> p m q", m=2),
                        )

            # unified pipeline: 32 heads; st0 norms + st0 out-proj wavefront
            # and deferred v blocks 3,4 fill supertile boundaries. st0's last
            # two out-proj columns are held back as PE filler spanning the
            # st1 reciprocal, so the PE never idles into a HAM re-throttle.
            po_queue = [(m, k) for m in range(5) for k in range(8)]
            po_ptr = 0
            norm_emitted = -1
            # deferred phase-1 fillers: kT[4+m] must land before QK(0, 8+2m)
            # reads it (step 8+2m), qT[4+m] likewise — both comfortably ahead
            fillers = {
                1: lambda: (emit_k_group(3, 0, 0, "v"), emit_k_group(3, 0, 1, "v")),
                2: lambda: (emit_k_group(3, 1, 0, "v"), emit_k_group(3, 1, 1, "v")),
                3: lambda: (emit_k_group(3, 2, 0, "v"), emit_k_group(3, 2, 1, "v")),
                4: lambda: (emit_k_group(3, 3, 0, "v"), emit_k_group(3, 3, 1, "v")),
                5: lambda: emit_q_group(1, 0, "v"),
                6: lambda: emit_q_group(1, 1, "v"),
                7: lambda: emit_q_group(1, 2, "v"),
                8: lambda: emit_q_group(1, 3, "v"),
                9: lambda: emit_v(3, 0, "v"),
                10: lambda: emit_v(3, 1, "v"),
                11: lambda: emit_v(4, 0, "v"),
                12: lambda: emit_v(4, 1, "v"),
            }
            for step in range(2 * H + DEPTH):
                if step in fillers:
                    fillers[step]()
                if step < 2 * H:
                    emit_qk(step // H, step % H)
                s = step - DEPTH
                if s >= 0:
                    emit_av(s // H, s % H)
                    if s == H - 1:
                        emit_recip(0)
                ns = step - (H + DEPTH)
                if 0 <= ns < 8:
                    emit_norm(0, 2 * ns)
                    emit_norm(0, 2 * ns + 1)
                    norm_emitted = 2 * ns + 1
                if step > H + DEPTH:
                    drained = 0
                    while po_ptr < 40 and drained < 6:
                        m, k = po_queue[po_ptr]
                        if 2 * k + 1 <= norm_emitted:
                            emit_po_unit(0, m, k)
                            po_ptr += 1
                            drained += 1
                        else:
                            break
            while po_ptr < 40:
                emit_po_unit(0, *po_queue[po_ptr])
                po_ptr += 1
            emit_recip(1)
            for m in (5, 6, 7):               # PE filler while DVE runs recip(1)
                for k in range(8):
                    emit_po_unit(0, m, k)
            po_queue = [(m, k) for m in range(8) for k in range(8)]
            po_ptr = 0
            for j in range(8):
                emit_norm(1, 2 * j)
                emit_norm(1, 2 * j + 1)
                while po_ptr < 64:
                    m, k = po_queue[po_ptr]
                    if k <= j:
                        emit_po_unit(1, m, k)
                        po_ptr += 1
                    else:
                        break
            while po_ptr < 64:
                emit_po_unit(1, *po_queue[po_ptr])
                po_ptr += 1

    nc.finalize()
    return nc


def _get_nc():
    if "nc" not in _cached:
        _cached["nc"] = _build()
    return _cached["nc"]


def _core_inputs(x, w_qkv, w_out):
    # pre-tile weights so each on-device DMA is one contiguous DRAM block:
    # w_qkv -> [cb, p, k*512+c] with w[k*128+p, cb*512+c];
    # w_out -> [p, k*1024+c] with w[k*128+p, c]
    wq_b = np.ascontiguousarray(
        w_qkv.astype(BF16).reshape(8, 128, 6, 512).transpose(2, 1, 0, 3).reshape(6, 128, 8 * 512)
    )
    wo_b = np.ascontiguousarray(
        w_out.astype(BF16).reshape(8, 128, 1024).transpose(1, 0, 2).reshape(128, 8 * 1024)
    )
    # one-hot selector: row h broadcast source for head h
    sel = np.zeros((16, 16 * 128), dtype=BF16)
    for h in range(H):
        sel[h, h * 128:(h + 1) * 128] = 1
    in_maps = []
    for c in range(NCORES):
        b, qs = c // CPB, (c % CPB) * SLOC
        xs = np.zeros((TLOC, D), dtype=np.float32)
        lo = max(0, qs - HALO)
        xs[HALO - (qs - lo):] = x[b, lo:qs + SLOC]
        # multiplicative 0/1 mask applied to exp(scores) on GpSimd.
        # mask[st][:, 0:128] covers [r0 x queries 0:128], [:, 128:256] covers
        # [r2 x queries 128:256], [:, 256:512] is r1 for all 256 queries.
        i = np.arange(256)[None, None, None, :]
        j = np.arange(128)[None, None, :, None]
        st = np.arange(NST)[:, None, None, None]
        r = np.arange(3)[None, :, None, None]
        qg = qs + st * 256 + i
        kg = qs + st * 256 - HALO + r * 128 + j
        allowed = (kg <= qg) & (kg > qg - WINDOW) & (kg >= 0)
        # additive bias on raw scores (exp applies the 0.125 scale)
        m3 = np.where(allowed, 0.0, -8e30).astype(np.float32)
        mask = np.empty((NST, 128, 512), dtype=np.float32)
        mask[:, :, 0:128] = m3[:, 0, :, 0:128]
        mask[:, :, 128:256] = m3[:, 2, :, 128:256]
        mask[:, :, 256:512] = m3[:, 1]
        in_maps.append(
            {
                "xT": np.ascontiguousarray(xs.T.astype(BF16)),
                "w_qkv": wq_b,
                "w_out": wo_b,
                "mask": mask.astype(BF16),
                "sel": sel,
                "eye": np.eye(128, dtype=BF16),
            }
        )
    return in_maps


def kernel(x, w_qkv, w_out, _trace=False, _trace_kwargs=None):
    from concourse.bass_utils import run_bass_kernel_spmd

    x = np.asarray(x, dtype=np.float32)
    w_qkv = np.asarray(w_qkv, dtype=np.float32)
    w_out = np.asarray(w_out, dtype=np.float32)
    nc = _get_nc()
    in_maps = _core_inputs(x, w_qkv, w_out)
    res = run_bass_kernel_spmd(
        nc, in_maps, list(range(NCORES)), trace=_trace, **(_trace_kwargs or {})
    )
    out = np.concatenate(
        [res.results[c]["outT"].astype(np.float32).T for c in range(NCORES)], axis=0
    ).reshape(B, S, D)
    if _trace:
        return out, res
    return out
